# revision 51
# baseline (speedup 1.0000x reference)
"""Trainium2 Bass kernel for localized 3x3-window multi-head attention. v4

Problem: B=8, N=4096 (64x64 grid), DIM=512, 8 heads x 64 dim, KSIZE=3.
  qkv = x @ w_qkv; per-head localized attention over zero-padded 3x3
  spatial neighborhood; out = attn_out @ w_out + b_out.

Sharding: data-parallel over batch - core i computes batch i (8 cores).

v4 over v3:
  - V / attn_out use (d, h) column order (w_qkv V-columns and w_out rows
    are host-permuted, mathematically identity). The AV multiply then
    reads attention weights through a middle-axis stride-0 broadcast AP
    [128, 64(bcast), 8] at full 2x bf16 DVE rate - the 9 ACT broadcast
    expansions (AX) are deleted.
  - exp() fused into the dots PSUM->SBUF copy (dsb = exp(dps), bf16).
  - mask/renorm folded in token-major on [128, 72] tiles.
  - output store bf16 (host casts to fp32; rel tol 2e-2).
  - four of the 9 AV mul ops run on GpSimd (Pool) to offload DVE.

v5/v6 over v4:
  - bias add folded into MM2 as a K=1 ones matmul (DVE add deleted);
    ACT copies MM2 PSUM to bf16 SBUF, out DMA via HWDGE (sync queue).
  - vdr scratch writes via sync queue too (no cast -> no gpsimd needed);
    Pool engine freed of ~1us/DMA software descriptor generation.
  - wrap mask applied as a -1000 logit-bias K=1 matmul accumulated into
    the dots PSUM (exp then yields 0 there); the [128,72] wm multiply on
    DVE is deleted. Z still gets +nw for the reference's zero-pad
    exp(0)=1 mass.
  - chunk-k halo copy on ACT; input DMAs issued in need order and
    split per chunk.

v9 (tile pairs):
  - attention runs on 256-token tile pairs: the 9-shift q*k products
    (P9) and the AV multiply-accumulate are double-width DVE/Pool ops
    (half the per-op fixed cost on the bottleneck vector engine);
    dots/softmax/MM2 stay per 128-token tile. Pair (4C, 4C+1) needs
    only chunk C, pair (4C+2, 4C+3) runs after mm1(C+1) fills the k
    halo / first v rows.
  - AV split tuned: 4 of 9 kk muls on GpSimd (Pool), 5 muls and all
    adds on DVE (shorter Pool serial burst per pair beat a 5/4 split).

v10 (merged pair-half dots):
  - both halves' ones-matmuls accumulate into one [16, 384] PSUM tile
    per s-group via widened per-(half, block) selector weights (half u
    -> psum rows 8u+2b+h2). Halves the exp copies, wrap-mask matmuls
    and tiny transposes, and merges the softmax (Z reduce / recip /
    normalize) into single 16-wide DVE ops for the whole pair. Each
    block's ones-matmuls are emitted right after its P92 product so the
    dots contraction starts as soon as block 0's products land.
  - buffer rebalance: 4 v9-window bufs (latency-critical DRAM
    roundtrip) paid for by 1 P92 buf (released early in each pair).

v12-v14 (engine rebalance; sim 252us -> 226us):
  - AV split retuned: 5 of 9 kk muls AND their 4 combining adds run on
    GpSimd (Pool, was 4 muls / 0 adds); DVE keeps 4 muls + 4 adds (DVE
    busy 216us -> 169us, Pool 55us -> 123us, both below PE's 189us).
  - x loads batched (2 DMAs per block: chunk 0, then cols 512..4096) and
    V stores batched (one [128, 4, DIM] staging tile + one DMA per
    chunk): SP DMA instruction count 235 -> 153.
  - b_out add fused into the MM2 PSUM->SBUF evict as a DVE tensor_add
    against a pre-broadcast [128, DIM] bias tile (built once via a K=1
    ones matmul); deletes the per-half bias matmul and the ACT evict.
  - v9 window bufs 4 -> 3 paying for the extra Pool partial tile.

v16:
  - dtok staged through SBUF by an ACT copy; the softmax Z-reduce and
    Ab2 normalize then read bf16 SBUF at DVE 2x instead of paying the
    PSUM-source 1x penalty (ACT has slack; DVE is the co-bottleneck).

v17:
  - drain-aware AV split: the last two pairs (t >= 28, after the final
    mm1, where the per-pair chain is fully exposed) use a balanced 4/5
    Pool/DVE split that shortens the serial Pool burst; steady-state
    pairs keep the throughput-optimal 5/4 split. Fill-side balancing
    was tried and hurts (DVE is busy with the next pairs' P9 products).

v18:
  - drain-aware P9 split: the last two pairs also send P9 products for
    blocks 2-3 to Pool (2/2 with DVE beats 3/1: the dots matmuls consume
    blocks in order, so early-blocks-on-fast-DVE overlaps best), further
    shortening the exposed product phase.
"""

import numpy as np
import ml_dtypes

bf16 = ml_dtypes.bfloat16

B, N, DIM = 8, 4096, 512
HEADS, HEAD_DIM, K9 = 8, 64, 9
GRID = 64          # 64x64 spatial grid
PAD = 65           # max |token shift| = 64+1
NT = N // 128      # 32 token tiles per core
NC4 = N // 512     # 8 512-token chunks per core
VROWS = N + 2 * PAD
HALO = 2 * PAD + 512   # haloed token window per chunk (642)

_CACHE = {}
_TRACE = False
_LAST_RESULTS = None


def _build(bias_zero=False):
    import concourse.bass as bass
    import concourse.mybir as mybir
    import concourse.tile as tile
    from concourse import bacc
    from concourse.bass import ts
    from concourse.masks import make_identity

    fp32 = mybir.dt.float32
    b16 = mybir.dt.bfloat16
    Copy = mybir.ActivationFunctionType.Copy
    Exp = mybir.ActivationFunctionType.Exp

    nc = bacc.Bacc("TRN2", target_bir_lowering=False, debug=False)

    xT = nc.dram_tensor("xT", [DIM, N], b16, kind="ExternalInput")
    wq = nc.dram_tensor("wq", [DIM, 3 * DIM], b16, kind="ExternalInput")
    wo = nc.dram_tensor("wo", [DIM, DIM], b16, kind="ExternalInput")
    bby = nc.dram_tensor("bby", [1, DIM], b16, kind="ExternalInput")
    # wrap logit-bias (0 / -1000), [(s a) to] layout; wrap count
    wmask = nc.dram_tensor("wmask", [1, K9 * 128], b16, kind="ExternalInput")
    nw = nc.dram_tensor("nw", [128, 1], fp32, kind="ExternalInput")
    out = nc.dram_tensor("out", [N, DIM], b16, kind="ExternalOutput")

    with tile.TileContext(nc) as tc:
        with (
            tc.tile_pool(name="const", bufs=1) as const,
            tc.tile_pool(name="dram", bufs=1, space="DRAM") as dpool,
            tc.tile_pool(name="qkt", bufs=2) as qktpool,
            tc.tile_pool(name="vs", bufs=2) as vspool,
            tc.tile_pool(name="v9", bufs=3) as v9pool,
            tc.tile_pool(name="attn", bufs=2) as apool,
            tc.tile_pool(name="p9p", bufs=1) as p9pool,
            tc.tile_pool(name="outp", bufs=3) as opool,
            tc.tile_pool(name="psqk", bufs=2, space="PSUM") as psqk,
            tc.tile_pool(name="psd", bufs=1, space="PSUM") as psd,
            tc.tile_pool(name="psdt", bufs=1, space="PSUM") as psdt,
            tc.tile_pool(name="pstp", bufs=1, space="PSUM") as pstp,
            tc.tile_pool(name="psm2", bufs=1, space="PSUM") as psm2,
        ):
            # ---- constants ----
            xT_sb = [const.tile([128, N], b16, name=f"xT{c}") for c in range(4)]
            wq_sb = [const.tile([128, 3 * DIM], b16, name=f"wq{c}") for c in range(4)]
            wo_sb = [const.tile([128, DIM], b16, name=f"wo{c}") for c in range(4)]
            # issue order = need order: wq, then x chunk 0, then the rest
            for c in range(4):
                nc.sync.dma_start(out=wq_sb[c], in_=wq[ts(c, 128), :])
            for c in range(4):
                nc.sync.dma_start(out=xT_sb[c][:, 0:512],
                                  in_=xT[ts(c, 128), 0:512])
            for c in range(4):
                nc.sync.dma_start(out=xT_sb[c][:, 512:N],
                                  in_=xT[ts(c, 128), 512:N])
            for c in range(4):
                nc.sync.dma_start(out=wo_sb[c], in_=wo[ts(c, 128), :])
            bby_sb = const.tile([1, DIM], b16, name="bby")
            nc.sync.dma_start(out=bby_sb, in_=bby[:, :])
            wm_sb = const.tile([1, K9 * 128], b16, name="wm")
            nc.sync.dma_start(out=wm_sb, in_=wmask[:, :])
            nw_sb = const.tile([128, 1], fp32, name="nw")
            nc.sync.dma_start(out=nw_sb, in_=nw[:, :])
            ones1 = const.tile([1, 128], b16, name="ones1")
            nc.vector.memset(ones1, 1.0)
            ident = const.tile([128, 128], b16, name="ident")
            make_identity(nc, ident)
            zero_sb = const.tile([128, DIM], b16, name="zero")
            nc.vector.memset(zero_sb, 0.0)
            bby_full = const.tile([128, DIM], b16, name="bbyf")
            # per-(half, block) ones weights: block b of pair-half u maps its
            # two 64-partition head segments to psum rows 8u+2b / 8u+2b+1
            # (other columns zero, so PSUM accumulation stacks all 8 MMs of
            # a pair into one [16, n] tile)
            onesb = []
            for u in range(2):
                row = []
                for b in range(4):
                    ob = const.tile([128, 16], b16, name=f"onesw{u}{b}")
                    nc.vector.memset(ob, 0.0)
                    for h2 in range(2):
                        col = 8 * u + 2 * b + h2
                        nc.vector.memset(
                            ob[64 * h2:64 * (h2 + 1), col:col + 1], 1.0)
                    row.append(ob)
                onesb.append(row)

            if not bias_zero:
                psbb = psm2.tile([128, DIM], fp32, tag="mm2")
                nc.tensor.matmul(psbb, lhsT=ones1, rhs=bby_sb,
                                 start=True, stop=True)
                nc.scalar.activation(bby_full, psbb, Copy)

            # ---- V DRAM scratch with zero pad rows ----
            vdr = dpool.tile([VROWS, DIM], b16, name="vscratch")
            nc.sync.dma_start(out=vdr[0:PAD, :], in_=zero_sb[0:PAD, :])
            nc.sync.dma_start(out=vdr[PAD + N:VROWS, :], in_=zero_sb[0:PAD, :])

            qT_tiles = {}   # chunk -> [4 blocks] of [128, 512]
            kT_tiles = {}   # chunk -> [4 blocks] of [128, HALO]
            v9_tiles = {}   # tile -> [128, 3, 3, DIM]

            def mm1(C):
                """Chunk C (512 tokens): qT,kT head-major; V token-major."""
                qTb = [qktpool.tile([128, 512], b16, tag=f"qT{b}",
                                    name=f"qT{b}_{C}") for b in range(4)]
                kTb = [qktpool.tile([128, HALO], b16, tag=f"kT{b}",
                                    name=f"kT{b}_{C}") for b in range(4)]
                qT_tiles[C] = qTb
                kT_tiles[C] = kTb
                for b in range(4):
                    for which in range(2):  # 0 = q, 1 = k
                        ps = psqk.tile([128, 512], fp32, tag="qk")
                        mcol = which * DIM + b * 128
                        for c in range(4):
                            nc.tensor.matmul(
                                ps, lhsT=wq_sb[c][:, mcol:mcol + 128],
                                rhs=xT_sb[c][:, ts(C, 512)],
                                start=(c == 0), stop=(c == 3))
                        if which == 0:
                            # attention scale pre-folded into wq on host
                            nc.scalar.activation(qTb[b], ps, Copy)
                        else:
                            nc.scalar.activation(kTb[b][:, PAD:PAD + 512], ps,
                                                 Copy)
                            if C > 0:
                                # my first 65 tokens are C-1's right halo
                                nc.scalar.activation(
                                    kT_tiles[C - 1][b][:, PAD + 512:HALO],
                                    ps[:, 0:PAD], Copy)
                                # C-1's last 65 tokens are my left halo
                                nc.scalar.activation(
                                    kTb[b][:, 0:PAD],
                                    kT_tiles[C - 1][b][:, 512:512 + PAD], Copy)
                            else:
                                nc.vector.memset(kTb[b][:, 0:PAD], 0.0)
                            if C == NC4 - 1:
                                nc.vector.memset(kTb[b][:, PAD + 512:HALO], 0.0)
                # V token-major (d,h column order), per 128-token tile;
                # all 4 tiles staged in one SBUF tile, stored as one DMA
                vt4 = vspool.tile([128, 4, DIM], b16, tag="vst")
                for tt in range(4):
                    t = 4 * C + tt
                    psv_t = psqk.tile([128, DIM], fp32, tag="qk")
                    for c in range(4):
                        nc.tensor.matmul(
                            psv_t, lhsT=xT_sb[c][:, ts(t, 128)],
                            rhs=wq_sb[c][:, 2 * DIM:3 * DIM],
                            start=(c == 0), stop=(c == 3))
                    nc.scalar.activation(vt4[:, tt], psv_t, Copy)
                dst = bass.AP(
                    tensor=vdr.tensor,
                    offset=vdr.offset + (PAD + C * 512) * DIM,
                    ap=[[DIM, 128], [128 * DIM, 4], [1, DIM]])
                nc.sync.dma_start(out=dst, in_=vt4)

            def prefetch_v2(t):
                """Window fetch for the tile pair (t, t+1), one DMA per half
                (5-dim APs don't balance in the DMA lowering)."""
                v92 = v9pool.tile([128, 2, 3, 3, DIM], b16, tag="v9")
                for half in range(2):
                    src = bass.AP(
                        tensor=vdr.tensor,
                        offset=vdr.offset + (t + half) * 128 * DIM,
                        ap=[[DIM, 128], [64 * DIM, 3], [DIM, 3], [1, DIM]])
                    nc.sync.dma_start(out=v92[:, half], in_=src)
                v9_tiles[t] = v92

            def attn2(t):
                """Attention for the tile pair (t, t+1); t even, same chunk.
                P9 and AV run as double-width DVE/Pool ops (half the per-op
                fixed cost on the bottleneck engine); dots/softmax/MM2 stay
                per tile."""
                C, tt = t // 4, t % 4
                qTb, kTb = qT_tiles[C], kT_tiles[C]
                v92 = v9_tiles.pop(t)
                # all-9-shift products per 2-head block over 256 tokens;
                # each block's ones-matmuls are emitted right after its P92
                # so the dots contraction starts as soon as block 0 lands
                Ab2 = apool.tile([128, 2, K9, HEADS], b16, tag="Ab2",
                                 name=f"Ab2_{t}")
                dps = [psd.tile([16, 3 * 128], fp32, tag=f"d{s}",
                                name=f"dps{s}_{t}") for s in range(3)]
                for b in range(4):
                    P92 = p9pool.tile([128, K9, 256], b16, tag=f"P9{b}",
                                     name=f"P9{b}_{t}")
                    qsl = qTb[b][:, tt * 128:tt * 128 + 256]
                    qin = qsl.unsqueeze(1).broadcast_to((128, K9, 256))
                    kbase = kTb[b][:, 0:1]
                    kin = bass.AP(
                        tensor=kbase.tensor, offset=kbase.offset + tt * 128,
                        ap=[list(kbase.ap[0]), [64, 3], [1, 3], [1, 256]])
                    if t >= 28 and b >= 2:
                        # drain: Pool takes half the P9 products so the
                        # exposed serial product phase shortens
                        nc.gpsimd.tensor_mul(
                            P92.rearrange("p (a c) x -> p a c x", a=3),
                            qin, kin)
                    else:
                        nc.vector.tensor_mul(
                            P92.rearrange("p (a c) x -> p a c x", a=3),
                            qin, kin)
                    for u in range(2):
                        for s in range(3):
                            nc.tensor.matmul(
                                dps[s], lhsT=onesb[u][b],
                                rhs=P92[:, 3 * s:3 * (s + 1),
                                        u * 128:u * 128 + 128],
                                start=(u == 0 and b == 0), stop=False)
                # -1000 logit bias at grid-row-wrap positions (all 16 rows)
                for s in range(3):
                    nc.tensor.matmul(
                        dps[s], lhsT=ones1[:, 0:16],
                        rhs=wm_sb[:, s * 384:(s + 1) * 384],
                        start=False, stop=True)
                # dots -> exp -> SBUF bf16 (fused), tiny PE transposes to
                # token-major [kk, u, h]
                dsb = apool.tile([16, K9 * 128], b16, tag="dsb")
                for s in range(3):
                    nc.scalar.activation(dsb[:, s * 384:(s + 1) * 384],
                                         dps[s], Exp)
                dtok = psdt.tile([128, K9 * 16], b16, tag="dtok")
                dsb3 = dsb.rearrange("p (k x) -> p k x", k=K9)
                for kk in range(K9):
                    nc.tensor.transpose(dtok[:, ts(kk, 16)], dsb3[:, kk, :],
                                        ident[0:16, 0:16])
                # token-major softmax normalization over kk for both halves
                # (via an SBUF copy: PSUM-source DVE ops run at 1x)
                dts = apool.tile([128, K9 * 16], b16, tag="dts")
                nc.scalar.activation(dts, dtok, Copy)
                dtv = dts.rearrange("p (k u h) -> p k u h", k=K9, u=2)
                Z = apool.tile([128, 2 * HEADS], fp32, tag="Z")
                nc.vector.tensor_reduce(
                    Z.rearrange("p (u h) -> p u h", u=2),
                    dtv.rearrange("p k u h -> p u h k"),
                    axis=mybir.AxisListType.X, op=mybir.AluOpType.add)
                nc.vector.tensor_scalar_add(Z, Z, nw_sb)
                Zr = apool.tile([128, 2 * HEADS], b16, tag="Zr")
                with nc.allow_low_precision(reason="1/Z in bf16: 0.4% "
                                            "rel err vs 2e-2 tolerance"):
                    nc.vector.reciprocal(Zr, Z)
                Zrv = Zr.rearrange("p (u h) -> p u h", u=2)
                nc.vector.tensor_mul(
                    Ab2, dtv.rearrange("p k u h -> p u k h"),
                    Zrv.unsqueeze(2).broadcast_to((128, 2, K9, HEADS)))

                # AV for both tiles at once: v9 is (d, h) so Ab broadcasts
                # over d via a middle-axis stride-0 AP at full 2x bf16 DVE
                # rate. kk 4..8 muls go to GpSimd (Pool) to offload DVE.
                v9v = v92.rearrange("p u a c (d h) -> p u a c d h", h=HEADS)

                def abc(kk):
                    return Ab2[:, :, kk, :].unsqueeze(2).broadcast_to(
                        (128, 2, HEAD_DIM, HEADS))

                def vsl(kk):
                    return v9v[:, :, kk // 3, kk % 3, :, :]

                av = apool.tile([128, 2, HEAD_DIM, HEADS], b16, tag="av")
                Pv = apool.tile([128, 2, HEAD_DIM, HEADS], b16, tag="Pv")
                Pw = [apool.tile([128, 2, HEAD_DIM, HEADS], b16, tag=f"Pw{i}",
                                 name=f"Pw{i}_{t}")
                      for i in range(5)]
                # steady state: Pool takes 5 muls + 4 adds (max engine
                # throughput); in the drain (last two pairs, no mm1 work
                # left) a balanced 4/5 split shortens the exposed serial
                # Pool burst instead
                pool_kks = (4, 5, 6, 7) if t >= 28 else (4, 5, 6, 7, 8)
                dve_kks = (1, 2, 3, 8) if t >= 28 else (1, 2, 3)
                for i, kk in enumerate(pool_kks):
                    nc.gpsimd.tensor_mul(Pw[i], abc(kk), vsl(kk))
                nc.gpsimd.tensor_add(Pw[0], Pw[0], Pw[1])
                nc.gpsimd.tensor_add(Pw[2], Pw[2], Pw[3])
                if len(pool_kks) == 5:
                    nc.gpsimd.tensor_add(Pw[0], Pw[0], Pw[4])
                nc.gpsimd.tensor_add(Pw[0], Pw[0], Pw[2])
                nc.vector.tensor_mul(av, abc(0), vsl(0))
                for kk in dve_kks:
                    nc.vector.tensor_mul(Pv, abc(kk), vsl(kk))
                    nc.vector.tensor_add(av, av, Pv)
                nc.vector.tensor_add(av, av, Pw[0])

                for half in range(2):
                    avf = av[:, half].rearrange("p d h -> p (d h)")
                    tp = pstp.tile([128, DIM], b16, tag="tp")
                    for c in range(4):
                        nc.tensor.transpose(tp[:, ts(c, 128)],
                                            avf[:, ts(c, 128)], ident)
                    lhsT = opool.tile([128, DIM], b16, tag="lhsT")
                    nc.scalar.activation(lhsT, tp, Copy)
                    ps2 = psm2.tile([128, DIM], fp32, tag="mm2")
                    for c in range(4):
                        nc.tensor.matmul(ps2, lhsT=lhsT[:, ts(c, 128)],
                                         rhs=wo_sb[c],
                                         start=(c == 0), stop=(c == 3))
                    ot = opool.tile([128, DIM], b16, tag="ot")
                    if bias_zero:
                        nc.scalar.activation(ot, ps2, Copy)
                    else:
                        # + b_out fused into the PSUM->SBUF evict (DVE)
                        nc.vector.tensor_add(ot, ps2, bby_full)
                    nc.sync.dma_start(out=out[ts(t + half, 128), :], in_=ot)

            # chunk-level software pipeline over tile pairs. Pair (4C, 4C+1)
            # only needs chunk C (k windows stay inside it); pair (4C+2,
            # 4C+3) needs the right k-halo / first v rows of chunk C+1, so
            # it runs after mm1(C+1).
            for C in range(NC4 + 1):
                if C < NC4:
                    mm1(C)
                    prefetch_v2(4 * C)
                    if C >= 1:
                        prefetch_v2(4 * C - 2)
                        attn2(4 * C - 2)
                    attn2(4 * C)
                else:
                    prefetch_v2(4 * C - 2)
                    attn2(4 * C - 2)

    nc.compile()
    return nc


def _wrap_mask():
    # logit bias: -1000 where the dj=+-1 neighbor wraps around a grid row,
    # layout [kk, to] flattened to [1, K9*128]
    m = np.zeros((K9, 128), dtype=np.float32)
    for p in range(128):
        j = p % GRID
        for kk in range(K9):
            dj = kk % 3 - 1
            if (j == 0 and dj == -1) or (j == GRID - 1 and dj == 1):
                m[kk, p] = -1000.0
    return np.ascontiguousarray(m.reshape(1, K9 * 128).astype(bf16))


def prepare(x, w_qkv, w_out, b_out, h_img=64, w_img=64):
    """Host-side preprocessing: returns (in_maps, nc)."""
    assert int(h_img) == GRID and int(w_img) == GRID
    if "nc" not in _CACHE:
        _CACHE["nc"] = _build()
    nc = _CACHE["nc"]

    # permute V columns (h,d) -> (d,h); same perm on w_out rows.
    # perm[new_col] = old_col: new (d, h) <- old (h, d)
    perm = (np.arange(HEAD_DIM)[:, None] + HEAD_DIM * np.arange(HEADS)[None, :]
            ).reshape(-1)  # index: old col h*64+d at new pos d*8+h
    wqkv = np.ascontiguousarray(w_qkv, dtype=np.float32).copy()
    wqkv[:, 0:DIM] *= HEAD_DIM ** -0.5          # fold attention scale into q
    wqkv[:, 2 * DIM:3 * DIM] = wqkv[:, 2 * DIM:3 * DIM][:, perm]
    wout = np.ascontiguousarray(w_out, dtype=np.float32)[perm, :]

    wqh = np.ascontiguousarray(wqkv.astype(bf16))
    woh = np.ascontiguousarray(wout.astype(bf16))
    bby = np.ascontiguousarray(b_out.astype(bf16).reshape(1, DIM))
    wm = _wrap_mask()
    # 3 wrapped window entries (one per di) at each grid-row edge
    nw = np.zeros((128, 1), dtype=np.float32)
    nw[np.arange(128) % GRID == 0] = 3.0
    nw[np.arange(128) % GRID == GRID - 1] = 3.0
    in_maps = []
    for i in range(B):
        xTi = np.ascontiguousarray(x[i].T.astype(bf16))
        in_maps.append(dict(xT=xTi, wq=wqh, wo=woh, bby=bby, wmask=wm, nw=nw))
    return in_maps, nc


def finish(stacked_outs, inputs):
    """stacked_outs: {"out": [B, N, DIM]} -> full [B, N, DIM] fp32."""
    return np.ascontiguousarray(stacked_outs["out"].astype(np.float32))


def kernel(x, w_qkv, w_out, b_out, h_img=64, w_img=64):
    from concourse.bass_utils import run_bass_kernel_spmd

    in_maps, nc = prepare(x, w_qkv, w_out, b_out, h_img, w_img)
    global _LAST_RESULTS
    res = run_bass_kernel_spmd(nc, in_maps, core_ids=list(range(B)),
                               trace=_TRACE)
    _LAST_RESULTS = res
    out = np.stack([r["out"] for r in res.results], axis=0)
    return finish({"out": out}, None)

